# revision 5
# baseline (speedup 1.0000x reference)
"""Trainium2 Bass kernel: fc1+relu -> LSTM(H=32, T=200) -> fc2 on last hidden.

Data parallel over 8 NeuronCores: batch 4096 -> 512 per core (4 btiles x 128).

Layout strategy (batch on partitions for all elementwise work; all engine
costs scale with free-dim size only, so elementwise tensors are shaped
[128 partitions, small free]):
  - x is pre-transposed HOST-side to xt [1600, 512] fp16 (row = 8*t + ch,
    ch 0:5 = x, ch 5 = 1.0 carrying fc1 bias + ones column): the DMA lands
    it directly in [128, 512] chunks (16 steps x 8ch on partitions), so no
    on-chip transposes for fc1.
  - fc1: per 4-step group, one matmul with a 4-copy block-diagonal
    stationary w1bd4 [32,128] at tile row position 32m -> psum [128, 512]
    (rows 32w:32w+21 = step's h1aug^T, pre-relu). One DVE copy to fp16
    SBUF storage P4_g. Relu is folded into the per-step staging copy (max).
  - Recurrence per step: stationary L = STG_par [53, 512] fp16
    (rows 0:32 h^T, 32:53 h1aug^T), moving = wcomb [53, 128];
    4 matmuls (one per 128-batch tile) -> gates G [128b, 512=4x128g] psum.
    Gate cols per btile: [f|i|g|o] x 32, with f,i,o columns pre-scaled 0.5.
    One tanh ACT -> t4; ts (t4+1)*0.5 -> P (sigmoids); 3 tts for
    c' = sig_f*c + sig_i*tanh_g; tanh ACT -> TC; tt -> H = sig_o*TC;
    2 PE transposes H -> psum; DVE copy -> STG_nextpar rows 0:32;
    Pool ts-copy (with relu max) h1aug_{t+1} -> STG_nextpar rows 32:53.
  - Two independent batch streams (btiles {0,1}, {2,3}) interleave their
    serial chains across the engines.
"""

import sys
import numpy as np
from contextlib import ExitStack

sys.path.insert(0, "/opt/trn_rl_repo")
sys.path.insert(0, "/opt/pypackages")

import concourse.bass as bass
import concourse.bacc as bacc
import concourse.tile as tile
import concourse.mybir as mybir
from concourse import bass_utils
from concourse.masks import make_identity

F32 = mybir.dt.float32
FP16 = mybir.dt.float16
AF = mybir.ActivationFunctionType
ALU = mybir.AluOpType

H = 32
B = 4096
T = 200
C8 = 8
NCORES = 8
BL = B // NCORES  # 512
NBT = BL // 128  # 4
NXT = (C8 * T + 127) // 128  # 13 xt chunks of [128, 512] (16 steps each)
NG = T // 4  # 50 fc1 groups of 4 steps

# gate blocks within a btile's 128 gate columns: [f, i, g, o]
_TORCH_BASE = {0: 32, 1: 0, 2: 64, 3: 96}  # f,i,g,o -> torch row base


def prep_consts(fc1_w, fc1_b, w_ih, w_hh, b_ih, b_hh, fc2_w, fc2_b):
    perm = np.zeros(4 * H, dtype=np.int64)
    scol = np.zeros(4 * H, dtype=np.float32)
    for col in range(4 * H):
        blk, j = col // H, col % H
        perm[col] = _TORCH_BASE[blk] + j
        scol[col] = 1.0 if blk == 2 else 0.5  # g unscaled, f/i/o halved
    # wcomb [53, 128]: rows 0:32 h-weights (x0.5: stg carries 2h),
    # 32:52 h1-weights, row 52 bias
    wcomb = np.zeros((53, 128), np.float32)
    wcomb[0:32] = 0.5 * (scol[:, None] * w_hh[perm]).T
    wcomb[32:52] = (scol[:, None] * w_ih[perm]).T
    wcomb[52] = scol * (b_ih + b_hh)[perm]
    # w1bd4 [128, 128]: 4 identical 32-row copies (m=0..3); within a copy,
    # row 8w+c (w=step-in-group, c=channel) -> cols 32w+r of step w's h1aug
    w1bd4 = np.zeros((128, 128), np.float32)
    for m in range(4):
        for w in range(4):
            for c in range(5):
                w1bd4[32 * m + 8 * w + c, 32 * w : 32 * w + 20] = fc1_w[:, c]
            w1bd4[32 * m + 8 * w + 5, 32 * w : 32 * w + 20] = fc1_b
            w1bd4[32 * m + 8 * w + 5, 32 * w + 20] = 1.0
    fc2wT = np.ascontiguousarray(0.5 * fc2_w.T)  # [32, 2] (stg carries 2h)
    f16 = np.float16
    return dict(wcomb=wcomb.astype(f16), w1bd4=w1bd4.astype(f16),
                fc2wT=fc2wT.astype(f16))


def emit(tc, outs, ins):
    nc = tc.nc
    ctx = ExitStack()
    out_d = outs["out"]  # [512, 2] f32

    consts = ctx.enter_context(tc.tile_pool(name="consts", bufs=1))
    ident = consts.tile([128, 128], FP16, tag="ident")
    make_identity(nc, ident[:])
    wcomb = consts.tile([53, 128], FP16, tag="wcomb")
    nc.sync.dma_start(wcomb[:], ins["wcomb"][:, :])
    w1bd4 = consts.tile([128, 128], FP16, tag="w1bd4")
    nc.sync.dma_start(w1bd4[:], ins["w1bd4"][:, :])
    fc2w = consts.tile([32, 2], FP16, tag="fc2w")
    nc.sync.dma_start(fc2w[:], ins["fc2wT"][:, :])

    # ---------------- pools ----------------
    xt_pool = ctx.enter_context(tc.tile_pool(name="xt", bufs=1))
    p4_pool = ctx.enter_context(tc.tile_pool(name="p4", bufs=1))
    st_pool = ctx.enter_context(tc.tile_pool(name="st", bufs=1))
    wk = ctx.enter_context(tc.tile_pool(name="wk", bufs=4))
    ps_g = ctx.enter_context(tc.tile_pool(name="psg", bufs=2, space="PSUM"))
    ps_f = ctx.enter_context(tc.tile_pool(name="psf", bufs=1, space="PSUM"))
    ps_h = ctx.enter_context(tc.tile_pool(name="psh", bufs=3, space="PSUM"))

    # xt chunks: DMA the host-transposed x straight in
    xtd = ins["xt"]  # [1664, 512] fp16 (13*128 rows; tail zero-padded)
    xtt = []
    for c in range(NXT):
        xc = xt_pool.tile([128, 512], FP16, tag=f"xt{c}", name=f"xt_{c}")
        nc.sync.dma_start(xc[:], xtd[128 * c : 128 * (c + 1), :])
        xtt.append(xc)

    # fc1 storage: 50 groups x [128, 512] fp16 (pre-relu h1aug^T, 4 steps)
    p4t = [p4_pool.tile([128, 512], FP16, tag=f"p4_{g}", name=f"p4_{g}")
           for g in range(NG)]

    def emit_fc1_group(g):
        c, m = g // 4, g % 4
        fps = ps_f.tile([128, 512], F32, tag="fps", name=f"fps_{g}")
        nc.tensor.matmul(fps[:], w1bd4[32 * m : 32 * (m + 1), :],
                         xtt[c][32 * m : 32 * (m + 1), :],
                         start=True, stop=True, tile_position=(32 * m, 0))
        nc.scalar.activation(p4t[g][:], fps[:], AF.Relu)

    # recurrence state
    stg = [st_pool.tile([53, 512], FP16, tag=f"stg{p}", name=f"stg{p}")
           for p in range(2)]
    nc.vector.memset(stg[0][:], 0.0)
    Cst = st_pool.tile([128, 128], FP16, tag="C", name="Cst")
    nc.vector.memset(Cst[:], 0.0)

    # prologue: fc1 groups 0..2, then stage h1aug_0 into stg[0]
    for g in range(3):
        emit_fc1_group(g)
    nc.vector.tensor_copy(stg[0][32:53, :], p4t[0][0:21, :])

    # ---------------- recurrence ----------------
    # State per stream: t4/TC/H tiles are looked up by (s) from these dicts,
    # written by one sub-phase and read by a later one. Emission is skewed:
    # stream b runs half a step behind stream a so their serial chains
    # interleave on the in-order engine queues.
    cur = [dict() for _ in range(2)]

    def p0_mm(t, s):  # gates matmul (PE), gate-major psum scatter:
        # G free pos = 64*gateblk + 32*j + h  ->  tf/ti/tg/to contiguous
        par = t % 2
        G = ps_g.tile([128, 256], F32, tag=f"G{s}", name=f"G{s}_{t}")
        Gv = G[:].rearrange("p (B two h) -> p B two h", two=2, h=32)
        for j in range(2):
            k = 2 * s + j
            nc.tensor.matmul(Gv[:, :, j, :],
                             stg[par][:, 128 * k : 128 * (k + 1)],
                             wcomb[:], start=True, stop=True,
                             tile_position=(0, 0))
        cur[s]["G"] = G

    def p1_tanh(t, s):  # gate nonlinearity (ACT), one instruction
        t4 = wk.tile([128, 256], FP16, tag=f"t4{s}", name=f"t4{s}_{t}")
        nc.scalar.activation(t4[:], cur[s]["G"][:], AF.Tanh)
        cur[s]["t4"] = t4

    def p2_cell(t, s):  # c update: 3 stt (DVE), all contiguous [128,64]
        t4 = cur[s]["t4"]
        Cs = Cst[:, 64 * s : 64 * (s + 1)]
        U = wk.tile([128, 64], FP16, tag=f"U{s}", name=f"U{s}_{t}")
        nc.vector.scalar_tensor_tensor(U[:], t4[:, 0:64], 1.0, Cs,
                                       ALU.add, ALU.mult)
        V = wk.tile([128, 64], FP16, tag=f"V{s}", name=f"V{s}_{t}")
        nc.vector.scalar_tensor_tensor(V[:], t4[:, 64:128], 1.0,
                                       t4[:, 128:192], ALU.add, ALU.mult)
        nc.vector.scalar_tensor_tensor(Cs, U[:], 0.5, V[:],
                                       ALU.mult, ALU.add)

    def p3_h(t, s):  # per-stream tanh(c) (ACT) + H (DVE) — streams decoupled
        TC = wk.tile([128, 64], FP16, tag=f"TC{s}", name=f"TC{s}_{t}")
        nc.scalar.activation(TC[:], Cst[:, 64 * s : 64 * (s + 1)],
                             AF.Tanh, scale=0.5)
        Hs = wk.tile([128, 64], FP16, tag=f"H{s}", name=f"H{s}_{t}")
        nc.vector.scalar_tensor_tensor(
            Hs[:], cur[s]["t4"][:, 192:256], 1.0, TC[:], ALU.add, ALU.mult)
        cur[s]["H"] = Hs

    def p4_transpose(t, s):  # H -> H^T (PE)
        HT = ps_h.tile([32, 256], FP16, tag="HT", name=f"HT{s}_{t}")
        for j in range(2):
            nc.tensor.transpose(HT[:, 128 * j : 128 * (j + 1)],
                                cur[s]["H"][:, 32 * j : 32 * (j + 1)],
                                ident[:])
        cur[s]["HT"] = HT

    def p5_stage(t, s):  # stage h^T (DVE)
        parn = (t + 1) % 2
        cols = slice(256 * s, 256 * (s + 1))
        nc.vector.tensor_copy(stg[parn][0:32, cols], cur[s]["HT"][:])

    def stage_h1aug(t):  # h1aug^T for step t+1, on idle GpSimd (off-chain)
        if t + 1 < T:
            parn = (t + 1) % 2
            g1, m1 = (t + 1) // 4, (t + 1) % 4
            nc.gpsimd.tensor_copy(
                stg[parn][32:53, :],
                p4t[g1][32 * m1 : 32 * m1 + 21, :])

    for t in range(T):
        # Alternate which stream gets engine-queue priority each step so the
        # per-stage queuing penalty splits between streams instead of
        # stacking on one.
        f, g = (0, 1) if t % 2 == 0 else (1, 0)
        p0_mm(t, f)
        p1_tanh(t, f)
        p0_mm(t, g)
        p1_tanh(t, g)
        stage_h1aug(t)
        p2_cell(t, f)
        p3_h(t, f)
        p4_transpose(t, f)
        p5_stage(t, f)
        p2_cell(t, g)
        p3_h(t, g)
        p4_transpose(t, g)
        p5_stage(t, g)
        if t % 4 == 0 and t // 4 + 3 < NG:
            emit_fc1_group(t // 4 + 3)

    # ---------------- fc2 ----------------
    f2p = ps_h.tile([128, 8], F32, tag="HT", name="f2p")
    for k in range(NBT):
        nc.tensor.matmul(f2p[:, 2 * k : 2 * k + 2],
                         stg[0][0:32, 128 * k : 128 * (k + 1)], fc2w[:],
                         start=True, stop=True, tile_position=(0, 0))
    f2s = wk.tile([128, 8], F32, tag="f2s", name="f2s")
    nc.vector.tensor_copy(f2s[:], f2p[:])
    for k in range(NBT):
        nc.sync.dma_start(out_d[128 * k : 128 * (k + 1), :],
                          f2s[:, 2 * k : 2 * k + 2])
    ctx.close()


_CACHE = {}


def _build():
    if "nc" in _CACHE:
        return _CACHE["nc"]
    nc = bacc.Bacc("TRN2", target_bir_lowering=False, debug=False,
                   enable_asserts=False, num_devices=NCORES)
    ins = {
        "xt": nc.dram_tensor("xt", [NXT * 128, BL], FP16,
                             kind="ExternalInput").ap(),
        "wcomb": nc.dram_tensor("wcomb", [53, 128], FP16,
                                kind="ExternalInput").ap(),
        "w1bd4": nc.dram_tensor("w1bd4", [128, 128], FP16,
                                kind="ExternalInput").ap(),
        "fc2wT": nc.dram_tensor("fc2wT", [32, 2], FP16,
                                kind="ExternalInput").ap(),
    }
    outs = {"out": nc.dram_tensor("out", [BL, 2], F32,
                                  kind="ExternalOutput").ap()}
    with tile.TileContext(nc) as tc:
        emit(tc, outs, ins)
    nc.compile()
    _CACHE["nc"] = nc
    return nc


def make_in_maps(x, fc1_w, fc1_b, w_ih, w_hh, b_ih, b_hh, fc2_w, fc2_b):
    consts = prep_consts(fc1_w, fc1_b, w_ih, w_hh, b_ih, b_hh, fc2_w, fc2_b)
    in_maps = []
    for c in range(NCORES):
        xs = x[c * BL : (c + 1) * BL]  # [512, 200, 5] f32
        x8 = np.zeros((BL, T, C8), np.float16)
        x8[:, :, 0:5] = xs
        x8[:, :, 5] = 1.0
        xt = np.zeros((NXT * 128, BL), np.float16)
        xt[0 : C8 * T] = x8.reshape(BL, C8 * T).T
        in_maps.append({"xt": np.ascontiguousarray(xt), **consts})
    return in_maps


def kernel(x, fc1_w, fc1_b, w_ih, w_hh, b_ih, b_hh, fc2_w, fc2_b,
           trace=False):
    x = np.asarray(x, np.float32)
    args = [np.asarray(a, np.float32)
            for a in (fc1_w, fc1_b, w_ih, w_hh, b_ih, b_hh, fc2_w, fc2_b)]
    nc = _build()
    in_maps = make_in_maps(x, *args)
    res = bass_utils.run_bass_kernel_spmd(
        nc, in_maps, core_ids=list(range(NCORES)), trace=trace)
    out = np.concatenate([r["out"] for r in res.results], axis=0)
    out = out + args[7][None, :]
    if trace:
        kernel.last_results = res
    return out.astype(np.float32)



# revision 6
# speedup vs baseline: 1.3515x; 1.3515x over previous
"""Trainium2 Bass kernel: fc1+relu -> LSTM(H=32, T=200) -> fc2 on last hidden.

Data parallel over 8 NeuronCores: batch 4096 -> 512 per core (4 btiles x 128).

Layout strategy (batch on partitions for all elementwise work; all engine
costs scale with free-dim size only, so elementwise tensors are shaped
[128 partitions, small free]):
  - x is pre-transposed HOST-side to xt [1600, 512] fp16 (row = 8*t + ch,
    ch 0:5 = x, ch 5 = 1.0 carrying fc1 bias + ones column): the DMA lands
    it directly in [128, 512] chunks (16 steps x 8ch on partitions), so no
    on-chip transposes for fc1.
  - fc1: per 4-step group, one matmul with a 4-copy block-diagonal
    stationary w1bd4 [32,128] at tile row position 32m -> psum [128, 512]
    (rows 32w:32w+21 = step's h1aug^T, pre-relu). One DVE copy to fp16
    SBUF storage P4_g. Relu is folded into the per-step staging copy (max).
  - Recurrence per step: stationary L = STG_par [53, 512] fp16
    (rows 0:32 h^T, 32:53 h1aug^T), moving = wcomb [53, 128];
    4 matmuls (one per 128-batch tile) -> gates G [128b, 512=4x128g] psum.
    Gate cols per btile: [f|i|g|o] x 32, with f,i,o columns pre-scaled 0.5.
    One tanh ACT -> t4; ts (t4+1)*0.5 -> P (sigmoids); 3 tts for
    c' = sig_f*c + sig_i*tanh_g; tanh ACT -> TC; tt -> H = sig_o*TC;
    2 PE transposes H -> psum; DVE copy -> STG_nextpar rows 0:32;
    Pool ts-copy (with relu max) h1aug_{t+1} -> STG_nextpar rows 32:53.
  - Two independent batch streams (btiles {0,1}, {2,3}) interleave their
    serial chains across the engines.
"""

import sys
import numpy as np
from contextlib import ExitStack

sys.path.insert(0, "/opt/trn_rl_repo")
sys.path.insert(0, "/opt/pypackages")

import concourse.bass as bass
import concourse.bacc as bacc
import concourse.tile as tile
import concourse.mybir as mybir
from concourse import bass_utils
from concourse.masks import make_identity

F32 = mybir.dt.float32
FP16 = mybir.dt.float16
AF = mybir.ActivationFunctionType
ALU = mybir.AluOpType

H = 32
B = 4096
T = 200
C8 = 8
NCORES = 8
BL = B // NCORES  # 512
NBT = BL // 128  # 4
NXT = (C8 * T + 127) // 128  # 13 xt chunks of [128, 512] (16 steps each)
NG = T // 4  # 50 fc1 groups of 4 steps

# gate blocks within a btile's 128 gate columns: [f, i, g, o]
_TORCH_BASE = {0: 32, 1: 0, 2: 64, 3: 96}  # f,i,g,o -> torch row base


def prep_consts(fc1_w, fc1_b, w_ih, w_hh, b_ih, b_hh, fc2_w, fc2_b):
    perm = np.zeros(4 * H, dtype=np.int64)
    scol = np.zeros(4 * H, dtype=np.float32)
    for col in range(4 * H):
        blk, j = col // H, col % H
        perm[col] = _TORCH_BASE[blk] + j
        scol[col] = 1.0 if blk == 2 else 0.5  # g unscaled, f/i/o halved
    # wcomb [53, 128]: rows 0:32 h-weights (x0.5: stg carries 2h),
    # 32:52 h1-weights, row 52 bias
    wcomb = np.zeros((53, 128), np.float32)
    wcomb[0:32] = 0.5 * (scol[:, None] * w_hh[perm]).T
    wcomb[32:52] = (scol[:, None] * w_ih[perm]).T
    wcomb[52] = scol * (b_ih + b_hh)[perm]
    # w1bd4 [128, 128]: 4 identical 32-row copies (m=0..3); within a copy,
    # row 8w+c (w=step-in-group, c=channel) -> cols 32w+r of step w's h1aug
    w1bd4 = np.zeros((128, 128), np.float32)
    for m in range(4):
        for w in range(4):
            for c in range(5):
                w1bd4[32 * m + 8 * w + c, 32 * w : 32 * w + 20] = fc1_w[:, c]
            w1bd4[32 * m + 8 * w + 5, 32 * w : 32 * w + 20] = fc1_b
            w1bd4[32 * m + 8 * w + 5, 32 * w + 20] = 1.0
    fc2wT = np.ascontiguousarray(0.5 * fc2_w.T)  # [32, 2] (stg carries 2h)
    f16 = np.float16
    return dict(wcomb=wcomb.astype(f16), w1bd4=w1bd4.astype(f16),
                fc2wT=fc2wT.astype(f16))


def emit(tc, outs, ins):
    nc = tc.nc
    ctx = ExitStack()
    out_d = outs["out"]  # [512, 2] f32

    consts = ctx.enter_context(tc.tile_pool(name="consts", bufs=1))
    ident = consts.tile([128, 128], FP16, tag="ident")
    make_identity(nc, ident[:])
    wcomb = consts.tile([53, 128], FP16, tag="wcomb")
    nc.sync.dma_start(wcomb[:], ins["wcomb"][:, :])
    w1bd4 = consts.tile([128, 128], FP16, tag="w1bd4")
    nc.sync.dma_start(w1bd4[:], ins["w1bd4"][:, :])
    fc2w = consts.tile([32, 2], FP16, tag="fc2w")
    nc.sync.dma_start(fc2w[:], ins["fc2wT"][:, :])

    # ---------------- pools ----------------
    xt_pool = ctx.enter_context(tc.tile_pool(name="xt", bufs=1))
    p4_pool = ctx.enter_context(tc.tile_pool(name="p4", bufs=1))
    st_pool = ctx.enter_context(tc.tile_pool(name="st", bufs=1))
    wk = ctx.enter_context(tc.tile_pool(name="wk", bufs=4))
    ps_g = ctx.enter_context(tc.tile_pool(name="psg", bufs=2, space="PSUM"))
    ps_f = ctx.enter_context(tc.tile_pool(name="psf", bufs=1, space="PSUM"))
    ps_h = ctx.enter_context(tc.tile_pool(name="psh", bufs=3, space="PSUM"))

    # xt chunks: DMA the host-transposed x straight in
    xtd = ins["xt"]  # [1664, 512] fp16 (13*128 rows; tail zero-padded)
    xtt = []
    for c in range(NXT):
        xc = xt_pool.tile([128, 512], FP16, tag=f"xt{c}", name=f"xt_{c}")
        nc.sync.dma_start(xc[:], xtd[128 * c : 128 * (c + 1), :])
        xtt.append(xc)

    # fc1 storage: 50 groups x [128, 512] fp16 (pre-relu h1aug^T, 4 steps)
    p4t = [p4_pool.tile([128, 512], FP16, tag=f"p4_{g}", name=f"p4_{g}")
           for g in range(NG)]

    def emit_fc1_group(g):
        c, m = g // 4, g % 4
        fps = ps_f.tile([128, 512], F32, tag="fps", name=f"fps_{g}")
        nc.tensor.matmul(fps[:], w1bd4[32 * m : 32 * (m + 1), :],
                         xtt[c][32 * m : 32 * (m + 1), :],
                         start=True, stop=True, tile_position=(32 * m, 0))
        nc.scalar.activation(p4t[g][:], fps[:], AF.Relu)

    # recurrence state
    stg = [st_pool.tile([53, 512], FP16, tag=f"stg{p}", name=f"stg{p}")
           for p in range(2)]
    nc.vector.memset(stg[0][:], 0.0)
    Cst = st_pool.tile([128, 128], FP16, tag="C", name="Cst")
    nc.vector.memset(Cst[:], 0.0)

    # prologue: fc1 groups 0..2, then stage h1aug_0 into stg[0]
    for g in range(3):
        emit_fc1_group(g)
    nc.vector.tensor_copy(stg[0][32:53, :], p4t[0][0:21, :])

    # ---------------- recurrence ----------------
    # State per stream: t4/TC/H tiles are looked up by (s) from these dicts,
    # written by one sub-phase and read by a later one. Emission is skewed:
    # stream b runs half a step behind stream a so their serial chains
    # interleave on the in-order engine queues.
    cur = [dict() for _ in range(2)]

    def p0_mm(t, s):  # gates matmul (PE), gate-major psum scatter:
        # G free pos = 64*gateblk + 32*j + h  ->  tf/ti/tg/to contiguous
        par = t % 2
        G = ps_g.tile([128, 256], F32, tag=f"G{s}", name=f"G{s}_{t}")
        Gv = G[:].rearrange("p (B two h) -> p B two h", two=2, h=32)
        for j in range(2):
            k = 2 * s + j
            nc.tensor.matmul(Gv[:, :, j, :],
                             stg[par][:, 128 * k : 128 * (k + 1)],
                             wcomb[:], start=True, stop=True,
                             tile_position=(0, 0))
        cur[s]["G"] = G

    def p1_tanh(t, s):  # gate nonlinearity (ACT), one instruction
        t4 = wk.tile([128, 256], FP16, tag=f"t4{s}", name=f"t4{s}_{t}")
        nc.scalar.activation(t4[:], cur[s]["G"][:], AF.Tanh)
        cur[s]["t4"] = t4

    def p2_cell(t, s):  # c update: 3 stt (DVE), all contiguous [128,64]
        t4 = cur[s]["t4"]
        Cs = Cst[:, 64 * s : 64 * (s + 1)]
        U = wk.tile([128, 64], FP16, tag=f"U{s}", name=f"U{s}_{t}")
        nc.vector.scalar_tensor_tensor(U[:], t4[:, 0:64], 1.0, Cs,
                                       ALU.add, ALU.mult)
        V = wk.tile([128, 64], FP16, tag=f"V{s}", name=f"V{s}_{t}")
        nc.vector.scalar_tensor_tensor(V[:], t4[:, 64:128], 1.0,
                                       t4[:, 128:192], ALU.add, ALU.mult)
        nc.vector.scalar_tensor_tensor(Cs, U[:], 0.5, V[:],
                                       ALU.mult, ALU.add)

    def p3_h(t, s):  # per-stream tanh(c) (ACT) + H (DVE) — streams decoupled
        TC = wk.tile([128, 64], FP16, tag=f"TC{s}", name=f"TC{s}_{t}")
        nc.scalar.activation(TC[:], Cst[:, 64 * s : 64 * (s + 1)],
                             AF.Tanh, scale=0.5)
        Hs = wk.tile([128, 64], FP16, tag=f"H{s}", name=f"H{s}_{t}")
        nc.vector.scalar_tensor_tensor(
            Hs[:], cur[s]["t4"][:, 192:256], 1.0, TC[:], ALU.add, ALU.mult)
        cur[s]["H"] = Hs

    def p4_transpose(t, s):  # H -> H^T (PE)
        HT = ps_h.tile([32, 256], FP16, tag="HT", name=f"HT{s}_{t}")
        for j in range(2):
            nc.tensor.transpose(HT[:, 128 * j : 128 * (j + 1)],
                                cur[s]["H"][:, 32 * j : 32 * (j + 1)],
                                ident[:])
        cur[s]["HT"] = HT

    def p5_stage(t, s):  # stage h^T (DVE)
        parn = (t + 1) % 2
        cols = slice(256 * s, 256 * (s + 1))
        nc.vector.tensor_copy(stg[parn][0:32, cols], cur[s]["HT"][:])

    def stage_h1aug(t):  # h1aug^T for step t+1 (DVE, during the tanh window)
        if t + 1 < T:
            parn = (t + 1) % 2
            g1, m1 = (t + 1) // 4, (t + 1) % 4
            nc.vector.tensor_copy(
                stg[parn][32:53, :],
                p4t[g1][32 * m1 : 32 * m1 + 21, :])

    for t in range(T):
        # Alternate which stream gets engine-queue priority each step so the
        # per-stage queuing penalty splits between streams instead of
        # stacking on one.
        f, g = (0, 1) if t % 2 == 0 else (1, 0)
        p0_mm(t, f)
        p1_tanh(t, f)
        p0_mm(t, g)
        p1_tanh(t, g)
        stage_h1aug(t)
        p2_cell(t, f)
        p3_h(t, f)
        p4_transpose(t, f)
        p5_stage(t, f)
        p2_cell(t, g)
        p3_h(t, g)
        p4_transpose(t, g)
        p5_stage(t, g)
        if t % 4 == 0 and t // 4 + 3 < NG:
            emit_fc1_group(t // 4 + 3)

    # ---------------- fc2 ----------------
    f2p = ps_h.tile([128, 8], F32, tag="HT", name="f2p")
    for k in range(NBT):
        nc.tensor.matmul(f2p[:, 2 * k : 2 * k + 2],
                         stg[0][0:32, 128 * k : 128 * (k + 1)], fc2w[:],
                         start=True, stop=True, tile_position=(0, 0))
    f2s = wk.tile([128, 8], F32, tag="f2s", name="f2s")
    nc.vector.tensor_copy(f2s[:], f2p[:])
    for k in range(NBT):
        nc.sync.dma_start(out_d[128 * k : 128 * (k + 1), :],
                          f2s[:, 2 * k : 2 * k + 2])
    ctx.close()


_CACHE = {}


def _build():
    if "nc" in _CACHE:
        return _CACHE["nc"]
    nc = bacc.Bacc("TRN2", target_bir_lowering=False, debug=False,
                   enable_asserts=False, num_devices=NCORES)
    ins = {
        "xt": nc.dram_tensor("xt", [NXT * 128, BL], FP16,
                             kind="ExternalInput").ap(),
        "wcomb": nc.dram_tensor("wcomb", [53, 128], FP16,
                                kind="ExternalInput").ap(),
        "w1bd4": nc.dram_tensor("w1bd4", [128, 128], FP16,
                                kind="ExternalInput").ap(),
        "fc2wT": nc.dram_tensor("fc2wT", [32, 2], FP16,
                                kind="ExternalInput").ap(),
    }
    outs = {"out": nc.dram_tensor("out", [BL, 2], F32,
                                  kind="ExternalOutput").ap()}
    with tile.TileContext(nc) as tc:
        emit(tc, outs, ins)
    nc.compile()
    _CACHE["nc"] = nc
    return nc


def make_in_maps(x, fc1_w, fc1_b, w_ih, w_hh, b_ih, b_hh, fc2_w, fc2_b):
    consts = prep_consts(fc1_w, fc1_b, w_ih, w_hh, b_ih, b_hh, fc2_w, fc2_b)
    in_maps = []
    for c in range(NCORES):
        xs = x[c * BL : (c + 1) * BL]  # [512, 200, 5] f32
        x8 = np.zeros((BL, T, C8), np.float16)
        x8[:, :, 0:5] = xs
        x8[:, :, 5] = 1.0
        xt = np.zeros((NXT * 128, BL), np.float16)
        xt[0 : C8 * T] = x8.reshape(BL, C8 * T).T
        in_maps.append({"xt": np.ascontiguousarray(xt), **consts})
    return in_maps


def kernel(x, fc1_w, fc1_b, w_ih, w_hh, b_ih, b_hh, fc2_w, fc2_b,
           trace=False):
    x = np.asarray(x, np.float32)
    args = [np.asarray(a, np.float32)
            for a in (fc1_w, fc1_b, w_ih, w_hh, b_ih, b_hh, fc2_w, fc2_b)]
    nc = _build()
    in_maps = make_in_maps(x, *args)
    res = bass_utils.run_bass_kernel_spmd(
        nc, in_maps, core_ids=list(range(NCORES)), trace=trace)
    out = np.concatenate([r["out"] for r in res.results], axis=0)
    out = out + args[7][None, :]
    if trace:
        kernel.last_results = res
    return out.astype(np.float32)



# revision 12
# speedup vs baseline: 1.3815x; 1.0222x over previous
"""Trainium2 Bass kernel: fc1+relu -> LSTM(H=32, T=200) -> fc2 on last hidden.

Data parallel over 8 NeuronCores: batch 4096 -> 512 per core (4 btiles x 128).

Layout strategy (batch on partitions for all elementwise work; all engine
costs scale with free-dim size only, so elementwise tensors are shaped
[128 partitions, small free]):
  - x is pre-transposed HOST-side to xt [1600, 512] fp16 (row = 8*t + ch,
    ch 0:5 = x, ch 5 = 1.0 carrying fc1 bias + ones column): the DMA lands
    it directly in [128, 512] chunks (16 steps x 8ch on partitions), so no
    on-chip transposes for fc1.
  - fc1: per 4-step group, one matmul with a 4-copy block-diagonal
    stationary w1bd4 [32,128] at tile row position 32m -> psum [128, 512]
    (rows 32w:32w+21 = step's h1aug^T, pre-relu). One DVE copy to fp16
    SBUF storage P4_g. Relu is folded into the per-step staging copy (max).
  - Recurrence per step: stationary L = STG_par [53, 512] fp16
    (rows 0:32 h^T, 32:53 h1aug^T), moving = wcomb [53, 128];
    4 matmuls (one per 128-batch tile) -> gates G [128b, 512=4x128g] psum.
    Gate cols per btile: [f|i|g|o] x 32, with f,i,o columns pre-scaled 0.5.
    One tanh ACT -> t4; ts (t4+1)*0.5 -> P (sigmoids); 3 tts for
    c' = sig_f*c + sig_i*tanh_g; tanh ACT -> TC; tt -> H = sig_o*TC;
    2 PE transposes H -> psum; DVE copy -> STG_nextpar rows 0:32;
    Pool ts-copy (with relu max) h1aug_{t+1} -> STG_nextpar rows 32:53.
  - Two independent batch streams (btiles {0,1}, {2,3}) interleave their
    serial chains across the engines.
"""

import sys
import numpy as np
from contextlib import ExitStack

sys.path.insert(0, "/opt/trn_rl_repo")
sys.path.insert(0, "/opt/pypackages")

import concourse.bass as bass
import concourse.bacc as bacc
import concourse.tile as tile
import concourse.mybir as mybir
from concourse import bass_utils
from concourse.masks import make_identity

F32 = mybir.dt.float32
FP16 = mybir.dt.float16
AF = mybir.ActivationFunctionType
ALU = mybir.AluOpType

H = 32
B = 4096
T = 200
C8 = 8
NCORES = 8
BL = B // NCORES  # 512
NBT = BL // 128  # 4
NXT = (C8 * T + 127) // 128  # 13 xt chunks of [128, 512] (16 steps each)
NG = T // 4  # 50 fc1 groups of 4 steps

# gate blocks within a btile's 128 gate columns: [f, i, g, o]
_TORCH_BASE = {0: 32, 1: 0, 2: 64, 3: 96}  # f,i,g,o -> torch row base


def prep_consts(fc1_w, fc1_b, w_ih, w_hh, b_ih, b_hh, fc2_w, fc2_b):
    perm = np.zeros(4 * H, dtype=np.int64)
    scol = np.zeros(4 * H, dtype=np.float32)
    for col in range(4 * H):
        blk, j = col // H, col % H
        perm[col] = _TORCH_BASE[blk] + j
        scol[col] = 1.0 if blk == 2 else 0.5  # g unscaled, f/i/o halved
    # wcomb [53, 128]: rows 0:32 h-weights (x0.5: stg carries 2h),
    # 32:52 h1-weights, row 52 bias
    wcomb = np.zeros((53, 128), np.float32)
    wcomb[0:32] = 0.5 * (scol[:, None] * w_hh[perm]).T
    wcomb[32:52] = (scol[:, None] * w_ih[perm]).T
    wcomb[52] = scol * (b_ih + b_hh)[perm]
    # w1bd4 [128, 128]: 4 identical 32-row copies (m=0..3); within a copy,
    # row 8w+c (w=step-in-group, c=channel) -> cols 32w+r of step w's h1aug
    w1bd4 = np.zeros((128, 128), np.float32)
    for m in range(4):
        for w in range(4):
            for c in range(5):
                w1bd4[32 * m + 8 * w + c, 32 * w : 32 * w + 20] = fc1_w[:, c]
            w1bd4[32 * m + 8 * w + 5, 32 * w : 32 * w + 20] = fc1_b
            w1bd4[32 * m + 8 * w + 5, 32 * w + 20] = 1.0
    fc2wT = np.ascontiguousarray(0.5 * fc2_w.T)  # [32, 2] (stg carries 2h)
    f16 = np.float16
    return dict(wcomb=wcomb.astype(f16), w1bd4=w1bd4.astype(f16),
                fc2wT=fc2wT.astype(f16))


def emit(tc, outs, ins):
    nc = tc.nc
    ctx = ExitStack()
    out_d = outs["out"]  # [512, 2] f32

    consts = ctx.enter_context(tc.tile_pool(name="consts", bufs=1))
    ident = consts.tile([128, 128], FP16, tag="ident")
    make_identity(nc, ident[:])
    wcomb = consts.tile([53, 128], FP16, tag="wcomb")
    nc.sync.dma_start(wcomb[:], ins["wcomb"][:, :])
    w1bd4 = consts.tile([128, 128], FP16, tag="w1bd4")
    nc.sync.dma_start(w1bd4[:], ins["w1bd4"][:, :])
    fc2w = consts.tile([32, 2], FP16, tag="fc2w")
    nc.sync.dma_start(fc2w[:], ins["fc2wT"][:, :])

    # ---------------- pools ----------------
    xt_pool = ctx.enter_context(tc.tile_pool(name="xt", bufs=1))
    p4_pool = ctx.enter_context(tc.tile_pool(name="p4", bufs=1))
    st_pool = ctx.enter_context(tc.tile_pool(name="st", bufs=1))
    wk = ctx.enter_context(tc.tile_pool(name="wk", bufs=4))
    ps_g = ctx.enter_context(tc.tile_pool(name="psg", bufs=1, space="PSUM"))
    ps_f = ctx.enter_context(tc.tile_pool(name="psf", bufs=1, space="PSUM"))
    ps_h = ctx.enter_context(tc.tile_pool(name="psh", bufs=2, space="PSUM"))

    # xt chunks: DMA the host-transposed x straight in
    xtd = ins["xt"]  # [1664, 512] fp16 (13*128 rows; tail zero-padded)
    xtt = []
    for c in range(NXT):
        xc = xt_pool.tile([128, 512], FP16, tag=f"xt{c}", name=f"xt_{c}")
        nc.sync.dma_start(xc[:], xtd[128 * c : 128 * (c + 1), :])
        xtt.append(xc)

    # fc1 storage: 50 groups x [128, 512] fp16 (pre-relu h1aug^T, 4 steps)
    p4t = [p4_pool.tile([128, 512], FP16, tag=f"p4_{g}", name=f"p4_{g}")
           for g in range(NG)]

    def emit_fc1_group(g):
        c, m = g // 4, g % 4
        fps = ps_f.tile([128, 512], F32, tag="fps", name=f"fps_{g}")
        nc.tensor.matmul(fps[:], w1bd4[32 * m : 32 * (m + 1), :],
                         xtt[c][32 * m : 32 * (m + 1), :],
                         start=True, stop=True, tile_position=(32 * m, 0))
        nc.scalar.activation(p4t[g][:], fps[:], AF.Relu)

    # recurrence state
    stg = [st_pool.tile([53, 512], FP16, tag=f"stg{p}", name=f"stg{p}")
           for p in range(2)]
    nc.vector.memset(stg[0][:], 0.0)
    Cst = [st_pool.tile([128, 64], FP16, tag=f"C{s}", name=f"Cst{s}")
           for s in range(2)]
    nc.vector.memset(Cst[0][:], 0.0)
    nc.vector.memset(Cst[1][:], 0.0)

    # prologue: fc1 groups 0..2, then stage h1aug_0 into stg[0]
    for g in range(3):
        emit_fc1_group(g)
    nc.vector.tensor_copy(stg[0][32:53, :], p4t[0][0:21, :])

    # ---------------- recurrence ----------------
    # State per stream: t4/TC/H tiles are looked up by (s) from these dicts,
    # written by one sub-phase and read by a later one. Emission is skewed:
    # stream b runs half a step behind stream a so their serial chains
    # interleave on the in-order engine queues.
    cur = [dict() for _ in range(2)]

    def p0_mm(t, s):  # gates matmul (PE), gate-major psum scatter:
        # G free pos = 64*gateblk + 32*j + h  ->  tf/ti/tg/to contiguous
        par = t % 2
        G = ps_g.tile([128, 256], F32, tag=f"G{s}", name=f"G{s}_{t}")
        Gv = G[:].rearrange("p (B two h) -> p B two h", two=2, h=32)
        for j in range(2):
            k = 2 * s + j
            nc.tensor.matmul(Gv[:, :, j, :],
                             stg[par][:, 128 * k : 128 * (k + 1)],
                             wcomb[:], start=True, stop=True,
                             tile_position=(0, 0))
        cur[s]["G"] = G

    def p1_tanh(t, s):  # gate nonlinearity (ACT), one instruction
        t4 = wk.tile([128, 256], FP16, tag=f"t4{s}", name=f"t4{s}_{t}")
        nc.scalar.activation(t4[:], cur[s]["G"][:], AF.Tanh)
        cur[s]["t4"] = t4

    def p2_cell(t, s):  # c update: 3 stt (DVE), all contiguous [128,64]
        t4 = cur[s]["t4"]
        Cs = Cst[s][:]
        U = wk.tile([128, 64], FP16, tag=f"U{s}", name=f"U{s}_{t}")
        nc.vector.scalar_tensor_tensor(U[:], t4[:, 0:64], 1.0, Cs,
                                       ALU.add, ALU.mult)
        V = wk.tile([128, 64], FP16, tag=f"V{s}", name=f"V{s}_{t}")
        nc.vector.scalar_tensor_tensor(V[:], t4[:, 64:128], 1.0,
                                       t4[:, 128:192], ALU.add, ALU.mult)
        nc.vector.scalar_tensor_tensor(Cs, U[:], 0.5, V[:],
                                       ALU.mult, ALU.add)

    def p3_h(t, s):  # per-stream tanh(c) (ACT) + H (DVE) — streams decoupled
        TC = wk.tile([128, 64], FP16, tag=f"TC{s}", name=f"TC{s}_{t}")
        nc.scalar.activation(TC[:], Cst[s][:], AF.Tanh, scale=0.5)
        Hs = wk.tile([128, 64], FP16, tag=f"H{s}", name=f"H{s}_{t}")
        nc.vector.scalar_tensor_tensor(
            Hs[:], cur[s]["t4"][:, 192:256], 1.0, TC[:], ALU.add, ALU.mult)
        cur[s]["H"] = Hs

    def p4_transpose(t, s):  # H -> H^T (PE)
        HT = ps_h.tile([32, 256], FP16, tag=f"HT{s}", name=f"HT{s}_{t}")
        for j in range(2):
            nc.tensor.transpose(HT[:, 128 * j : 128 * (j + 1)],
                                cur[s]["H"][:, 32 * j : 32 * (j + 1)],
                                ident[:])
        cur[s]["HT"] = HT

    def p5_stage(t, s):  # stage h^T (DVE)
        parn = (t + 1) % 2
        cols = slice(256 * s, 256 * (s + 1))
        nc.vector.tensor_copy(stg[parn][0:32, cols], cur[s]["HT"][:])

    def stage_h1aug(t):  # h1aug^T for step t+1 (DVE, during the tanh window)
        if t + 1 < T:
            parn = (t + 1) % 2
            g1, m1 = (t + 1) // 4, (t + 1) % 4
            nc.vector.tensor_copy(
                stg[parn][32:53, :],
                p4t[g1][32 * m1 : 32 * m1 + 21, :])

    for t in range(T):
        # Alternate which stream gets engine-queue priority each step so the
        # per-stage queuing penalty splits between streams instead of
        # stacking on one.
        f, g = (0, 1) if t % 2 == 0 else (1, 0)
        p0_mm(t, f)
        p1_tanh(t, f)
        p0_mm(t, g)
        p1_tanh(t, g)
        stage_h1aug(t)
        p2_cell(t, f)
        p3_h(t, f)
        p4_transpose(t, f)
        p5_stage(t, f)
        p2_cell(t, g)
        p3_h(t, g)
        p4_transpose(t, g)
        p5_stage(t, g)
        if t % 4 == 0 and t // 4 + 3 < NG:
            emit_fc1_group(t // 4 + 3)

    # ---------------- fc2 ----------------
    f2p = ps_f.tile([128, 8], F32, tag="f2p", name="f2p")
    for k in range(NBT):
        nc.tensor.matmul(f2p[:, 2 * k : 2 * k + 2],
                         stg[0][0:32, 128 * k : 128 * (k + 1)], fc2w[:],
                         start=True, stop=True, tile_position=(0, 0))
    f2s = wk.tile([128, 8], F32, tag="f2s", name="f2s")
    nc.vector.tensor_copy(f2s[:], f2p[:])
    for k in range(NBT):
        nc.sync.dma_start(out_d[128 * k : 128 * (k + 1), :],
                          f2s[:, 2 * k : 2 * k + 2])
    ctx.close()


_CACHE = {}


def _build():
    if "nc" in _CACHE:
        return _CACHE["nc"]
    nc = bacc.Bacc("TRN2", target_bir_lowering=False, debug=False,
                   enable_asserts=False, num_devices=NCORES)
    ins = {
        "xt": nc.dram_tensor("xt", [NXT * 128, BL], FP16,
                             kind="ExternalInput").ap(),
        "wcomb": nc.dram_tensor("wcomb", [53, 128], FP16,
                                kind="ExternalInput").ap(),
        "w1bd4": nc.dram_tensor("w1bd4", [128, 128], FP16,
                                kind="ExternalInput").ap(),
        "fc2wT": nc.dram_tensor("fc2wT", [32, 2], FP16,
                                kind="ExternalInput").ap(),
    }
    outs = {"out": nc.dram_tensor("out", [BL, 2], F32,
                                  kind="ExternalOutput").ap()}
    with tile.TileContext(nc) as tc:
        emit(tc, outs, ins)
    nc.compile()
    _CACHE["nc"] = nc
    return nc


def make_in_maps(x, fc1_w, fc1_b, w_ih, w_hh, b_ih, b_hh, fc2_w, fc2_b):
    consts = prep_consts(fc1_w, fc1_b, w_ih, w_hh, b_ih, b_hh, fc2_w, fc2_b)
    in_maps = []
    for c in range(NCORES):
        xs = x[c * BL : (c + 1) * BL]  # [512, 200, 5] f32
        x8 = np.zeros((BL, T, C8), np.float16)
        x8[:, :, 0:5] = xs
        x8[:, :, 5] = 1.0
        xt = np.zeros((NXT * 128, BL), np.float16)
        xt[0 : C8 * T] = x8.reshape(BL, C8 * T).T
        in_maps.append({"xt": np.ascontiguousarray(xt), **consts})
    return in_maps


def kernel(x, fc1_w, fc1_b, w_ih, w_hh, b_ih, b_hh, fc2_w, fc2_b,
           trace=False):
    x = np.asarray(x, np.float32)
    args = [np.asarray(a, np.float32)
            for a in (fc1_w, fc1_b, w_ih, w_hh, b_ih, b_hh, fc2_w, fc2_b)]
    nc = _build()
    in_maps = make_in_maps(x, *args)
    res = bass_utils.run_bass_kernel_spmd(
        nc, in_maps, core_ids=list(range(NCORES)), trace=trace)
    out = np.concatenate([r["out"] for r in res.results], axis=0)
    out = out + args[7][None, :]
    if trace:
        kernel.last_results = res
    return out.astype(np.float32)



# revision 27
# speedup vs baseline: 1.4233x; 1.0302x over previous
"""Trainium2 Bass kernel: fc1+relu -> LSTM(H=32, T=200) -> fc2 on last hidden.

Data parallel over 8 NeuronCores: batch 4096 -> 512 per core (4 btiles x 128).

Layout strategy (batch on partitions for all elementwise work; all engine
costs scale with free-dim size only, so elementwise tensors are shaped
[128 partitions, small free]):
  - x is pre-transposed HOST-side to xt [1600, 512] fp16 (row = 8*t + ch,
    ch 0:5 = x, ch 5 = 1.0 carrying fc1 bias + ones column): the DMA lands
    it directly in [128, 512] chunks (16 steps x 8ch on partitions), so no
    on-chip transposes for fc1.
  - fc1: per 4-step group, one matmul with a 4-copy block-diagonal
    stationary w1bd4 [32,128] at tile row position 32m -> psum [128, 512]
    (rows 32w:32w+21 = step's h1aug^T, pre-relu). One DVE copy to fp16
    SBUF storage P4_g. Relu is folded into the per-step staging copy (max).
  - Recurrence per step: stationary L = STG_par [53, 512] fp16
    (rows 0:32 h^T, 32:53 h1aug^T), moving = wcomb [53, 128];
    4 matmuls (one per 128-batch tile) -> gates G [128b, 512=4x128g] psum.
    Gate cols per btile: [f|i|g|o] x 32, with f,i,o columns pre-scaled 0.5.
    One tanh ACT -> t4; ts (t4+1)*0.5 -> P (sigmoids); 3 tts for
    c' = sig_f*c + sig_i*tanh_g; tanh ACT -> TC; tt -> H = sig_o*TC;
    2 PE transposes H -> psum; DVE copy -> STG_nextpar rows 0:32;
    Pool ts-copy (with relu max) h1aug_{t+1} -> STG_nextpar rows 32:53.
  - Two independent batch streams (btiles {0,1}, {2,3}) interleave their
    serial chains across the engines.
"""

import sys
import numpy as np
from contextlib import ExitStack

sys.path.insert(0, "/opt/trn_rl_repo")
sys.path.insert(0, "/opt/pypackages")

import concourse.bass as bass
import concourse.bacc as bacc
import concourse.tile as tile
import concourse.mybir as mybir
from concourse import bass_utils
from concourse.masks import make_identity

F32 = mybir.dt.float32
FP16 = mybir.dt.float16
AF = mybir.ActivationFunctionType
ALU = mybir.AluOpType

H = 32
B = 4096
T = 200
C8 = 8
NCORES = 8
BL = B // NCORES  # 512
NBT = BL // 128  # 4
NXT = (C8 * T + 127) // 128  # 13 xt chunks of [128, 512] (16 steps each)
NG = T // 4  # 50 fc1 groups of 4 steps

# gate blocks within a btile's 128 gate columns: [f, i, o, g]
_TORCH_BASE = {0: 32, 1: 0, 2: 96, 3: 64}  # f,i,o,g -> torch row base


def prep_consts(fc1_w, fc1_b, w_ih, w_hh, b_ih, b_hh, fc2_w, fc2_b):
    perm = np.zeros(4 * H, dtype=np.int64)
    scol = np.zeros(4 * H, dtype=np.float32)
    for col in range(4 * H):
        blk, j = col // H, col % H
        perm[col] = _TORCH_BASE[blk] + j
        scol[col] = 1.0 if blk == 3 else 0.5  # g unscaled, f/i/o halved
    # wch [32, 128]: h-weights; wcx4 [128, 128]: h1-weights + bias row,
    # replicated at partition offsets 32m so mmx fmap/weight offsets match
    wch = np.ascontiguousarray((scol[:, None] * w_hh[perm]).T)
    wcx = np.zeros((21, 128), np.float32)
    wcx[0:20] = (scol[:, None] * w_ih[perm]).T
    wcx[20] = scol * (b_ih + b_hh)[perm]
    wcx4 = np.zeros((128, 128), np.float32)
    for m in range(4):
        wcx4[32 * m : 32 * m + 21] = wcx
    # w1bd4 [128, 128]: 4 identical 32-row copies (m=0..3); within a copy,
    # row 8w+c (w=step-in-group, c=channel) -> cols 32w+r of step w's h1aug
    w1bd4 = np.zeros((128, 128), np.float32)
    for m in range(4):
        for w in range(4):
            for c in range(5):
                w1bd4[32 * m + 8 * w + c, 32 * w : 32 * w + 20] = fc1_w[:, c]
            w1bd4[32 * m + 8 * w + 5, 32 * w : 32 * w + 20] = fc1_b
            w1bd4[32 * m + 8 * w + 5, 32 * w + 20] = 1.0
    fc2wT = np.ascontiguousarray(fc2_w.T)  # [32, 2]
    f16 = np.float16
    wcomb = np.vstack([wch, wcx])  # [53, 128]
    return dict(wcomb=wcomb.astype(f16), w1bd4=w1bd4.astype(f16),
                fc2wT=fc2wT.astype(f16))


def emit(tc, outs, ins):
    nc = tc.nc
    ctx = ExitStack()
    out_d = outs["out"]  # [512, 2] f32

    consts = ctx.enter_context(tc.tile_pool(name="consts", bufs=1))
    ident = consts.tile([128, 128], FP16, tag="ident")
    make_identity(nc, ident[:])
    wcomb = consts.tile([53, 128], FP16, tag="wcomb")
    nc.sync.dma_start(wcomb[:], ins["wcomb"][:, :])
    w1bd4 = consts.tile([128, 128], FP16, tag="w1bd4")
    nc.sync.dma_start(w1bd4[:], ins["w1bd4"][:, :])
    fc2w = consts.tile([32, 2], FP16, tag="fc2w")
    nc.sync.dma_start(fc2w[:], ins["fc2wT"][:, :])

    # ---------------- pools ----------------
    xt_pool = ctx.enter_context(tc.tile_pool(name="xt", bufs=1))
    p4_pool = ctx.enter_context(tc.tile_pool(name="p4", bufs=1))
    st_pool = ctx.enter_context(tc.tile_pool(name="st", bufs=1))
    wk = ctx.enter_context(tc.tile_pool(name="wk", bufs=4))
    ps_g = ctx.enter_context(tc.tile_pool(name="psg", bufs=2, space="PSUM"))
    ps_f = ctx.enter_context(tc.tile_pool(name="psf", bufs=1, space="PSUM"))
    ps_h = ctx.enter_context(tc.tile_pool(name="psh", bufs=1, space="PSUM"))

    # xt chunks: DMA the host-transposed x straight in
    xtd = ins["xt"]  # [1664, 512] fp16 (13*128 rows; tail zero-padded)
    xtt = []
    for c in range(NXT):
        xc = xt_pool.tile([128, 512], FP16, tag=f"xt{c}", name=f"xt_{c}")
        nc.sync.dma_start(xc[:], xtd[128 * c : 128 * (c + 1), :])
        xtt.append(xc)

    # fc1 storage: 50 groups x [128, 512] fp16 (pre-relu h1aug^T, 4 steps)
    p4t = [p4_pool.tile([128, 512], FP16, tag=f"p4_{g}", name=f"p4_{g}")
           for g in range(NG)]

    def emit_fc1_group(g):
        c, m = g // 4, g % 4
        fps = ps_f.tile([128, 512], F32, tag="fps", name=f"fps_{g}")
        nc.tensor.matmul(fps[:], w1bd4[32 * m : 32 * (m + 1), :],
                         xtt[c][32 * m : 32 * (m + 1), :],
                         start=True, stop=True, tile_position=(32 * m, 0))
        nc.scalar.activation(p4t[g][:], fps[:], AF.Relu)

    # recurrence state: rows 0:32 h^T, rows 32:53 h1aug^T
    stg = [st_pool.tile([53, 512], FP16, tag=f"stg{p}", name=f"stg{p}")
           for p in range(2)]
    nc.vector.memset(stg[0][:], 0.0)
    Cst = [st_pool.tile([128, 64], FP16, tag=f"C{s}", name=f"Cst{s}")
           for s in range(2)]
    nc.vector.memset(Cst[0][:], 0.0)
    nc.vector.memset(Cst[1][:], 0.0)

    # prologue: fc1 groups 0..2, then stage h1aug_0
    for g in range(3):
        emit_fc1_group(g)
    nc.vector.tensor_copy(stg[0][32:53, :], p4t[0][0:21, :])

    # ---------------- recurrence ----------------
    # State per stream: t4/TC/H tiles are looked up by (s) from these dicts,
    # written by one sub-phase and read by a later one. Emission is skewed:
    # stream b runs half a step behind stream a so their serial chains
    # interleave on the in-order engine queues.
    cur = [dict() for _ in range(2)]

    def p0_mm(t, s):  # gates matmul (PE)
        par = t % 2
        G = ps_g.tile([128, 256], F32, tag=f"G{s}", name=f"G{s}_{t}")
        Gv = G[:].rearrange("p (B two h) -> p B two h", two=2, h=32)
        for j in range(2):
            k = 2 * s + j
            nc.tensor.matmul(Gv[:, :, j, :],
                             stg[par][:, 128 * k : 128 * (k + 1)],
                             wcomb[:], start=True, stop=True,
                             tile_position=(0, 0))
        cur[s]["G"] = G

    def stage_h1aug(t):  # h1aug^T for step t+1 (DVE, during tanh window)
        if t + 1 < T:
            parn = (t + 1) % 2
            g1, m1 = (t + 1) // 4, (t + 1) % 4
            nc.vector.tensor_copy(
                stg[parn][32:53, :],
                p4t[g1][32 * m1 : 32 * m1 + 21, :])

    def p1_tanh(t, s):  # gate nonlinearity (ACT), one instruction
        t4 = wk.tile([128, 256], FP16, tag=f"t4{s}", name=f"t4{s}_{t}")
        nc.scalar.activation(t4[:], cur[s]["G"][:], AF.Tanh)
        cur[s]["t4"] = t4

    def p2_cell(t, s):  # sigma-form cell: 1 ts (4x) + 3 tt (2x), [128,64]
        t4 = cur[s]["t4"]
        Cs = Cst[s][:]
        # SG = (t4_fio + 1) * 0.5 = [sig_f | sig_i | sig_o]  (cols 0:192)
        SG = wk.tile([128, 192], FP16, tag=f"SG{s}", name=f"SG{s}_{t}")
        nc.vector.tensor_scalar(SG[:], t4[:, 0:192], 1.0, 0.5,
                                ALU.add, ALU.mult)
        cur[s]["SG"] = SG
        U = wk.tile([128, 64], FP16, tag=f"U{s}", name=f"U{s}_{t}")
        nc.vector.tensor_tensor(U[:], SG[:, 0:64], Cs, ALU.mult)
        V = wk.tile([128, 64], FP16, tag=f"V{s}", name=f"V{s}_{t}")
        nc.vector.tensor_tensor(V[:], SG[:, 64:128], t4[:, 192:256],
                                ALU.mult)
        nc.vector.tensor_tensor(Cs, U[:], V[:], ALU.add)

    def p3_h(t, s):  # per-stream tanh(c) (ACT) + H (DVE) — streams decoupled
        TC = wk.tile([128, 64], FP16, tag=f"TC{s}", name=f"TC{s}_{t}")
        nc.scalar.activation(TC[:], Cst[s][:], AF.Tanh)
        Hs = wk.tile([128, 64], FP16, tag=f"H{s}", name=f"H{s}_{t}")
        nc.vector.tensor_tensor(Hs[:], cur[s]["SG"][:, 128:192], TC[:],
                                ALU.mult)
        cur[s]["H"] = Hs

    def p4_transpose(t, s):  # H -> H^T (PE)
        HT = ps_h.tile([32, 256], FP16, tag=f"HT{s}", name=f"HT{s}_{t}")
        for j in range(2):
            nc.tensor.transpose(HT[:, 128 * j : 128 * (j + 1)],
                                cur[s]["H"][:, 32 * j : 32 * (j + 1)],
                                ident[:])
        cur[s]["HT"] = HT

    def p5_stage(t, s):  # stage h^T (DVE)
        parn = (t + 1) % 2
        cols = slice(256 * s, 256 * (s + 1))
        nc.vector.tensor_copy(stg[parn][0:32, cols], cur[s]["HT"][:])

    for t in range(T):
        # Alternate which stream gets engine-queue priority each step so the
        # per-stage queuing penalty splits between streams instead of
        # stacking on one.
        f, g = (0, 1) if t % 2 == 0 else (1, 0)
        p0_mm(t, f)
        p1_tanh(t, f)
        p0_mm(t, g)
        p1_tanh(t, g)
        stage_h1aug(t)
        p2_cell(t, f)
        p3_h(t, f)
        p4_transpose(t, f)
        p5_stage(t, f)
        p2_cell(t, g)
        p3_h(t, g)
        p4_transpose(t, g)
        p5_stage(t, g)
        if t % 4 == 0 and t // 4 + 3 < NG:
            emit_fc1_group(t // 4 + 3)

    # ---------------- fc2 ----------------
    f2p = ps_f.tile([128, 8], F32, tag="f2p", name="f2p")
    for k in range(NBT):
        nc.tensor.matmul(f2p[:, 2 * k : 2 * k + 2],
                         stg[0][0:32, 128 * k : 128 * (k + 1)], fc2w[:],
                         start=True, stop=True, tile_position=(0, 0))
    f2s = wk.tile([128, 8], F32, tag="f2s", name="f2s")
    nc.vector.tensor_copy(f2s[:], f2p[:])
    for k in range(NBT):
        nc.sync.dma_start(out_d[128 * k : 128 * (k + 1), :],
                          f2s[:, 2 * k : 2 * k + 2])
    ctx.close()


_CACHE = {}


def _build():
    if "nc" in _CACHE:
        return _CACHE["nc"]
    nc = bacc.Bacc("TRN2", target_bir_lowering=False, debug=False,
                   enable_asserts=False, num_devices=NCORES)
    ins = {
        "xt": nc.dram_tensor("xt", [NXT * 128, BL], FP16,
                             kind="ExternalInput").ap(),
        "wcomb": nc.dram_tensor("wcomb", [53, 128], FP16,
                                kind="ExternalInput").ap(),
        "w1bd4": nc.dram_tensor("w1bd4", [128, 128], FP16,
                                kind="ExternalInput").ap(),
        "fc2wT": nc.dram_tensor("fc2wT", [32, 2], FP16,
                                kind="ExternalInput").ap(),
    }
    outs = {"out": nc.dram_tensor("out", [BL, 2], F32,
                                  kind="ExternalOutput").ap()}
    with tile.TileContext(nc) as tc:
        emit(tc, outs, ins)
    nc.compile()
    _CACHE["nc"] = nc
    return nc


def make_in_maps(x, fc1_w, fc1_b, w_ih, w_hh, b_ih, b_hh, fc2_w, fc2_b):
    consts = prep_consts(fc1_w, fc1_b, w_ih, w_hh, b_ih, b_hh, fc2_w, fc2_b)
    in_maps = []
    for c in range(NCORES):
        xs = x[c * BL : (c + 1) * BL]  # [512, 200, 5] f32
        x8 = np.zeros((BL, T, C8), np.float16)
        x8[:, :, 0:5] = xs
        x8[:, :, 5] = 1.0
        xt = np.zeros((NXT * 128, BL), np.float16)
        xt[0 : C8 * T] = x8.reshape(BL, C8 * T).T
        in_maps.append({"xt": np.ascontiguousarray(xt), **consts})
    return in_maps


def kernel(x, fc1_w, fc1_b, w_ih, w_hh, b_ih, b_hh, fc2_w, fc2_b,
           trace=False):
    x = np.asarray(x, np.float32)
    args = [np.asarray(a, np.float32)
            for a in (fc1_w, fc1_b, w_ih, w_hh, b_ih, b_hh, fc2_w, fc2_b)]
    nc = _build()
    in_maps = make_in_maps(x, *args)
    res = bass_utils.run_bass_kernel_spmd(
        nc, in_maps, core_ids=list(range(NCORES)), trace=trace)
    out = np.concatenate([r["out"] for r in res.results], axis=0)
    out = out + args[7][None, :]
    if trace:
        kernel.last_results = res
    return out.astype(np.float32)



# revision 31
# speedup vs baseline: 1.4260x; 1.0019x over previous
"""Trainium2 Bass kernel: fc1+relu -> LSTM(H=32, T=200) -> fc2 on last hidden.

Data parallel over 8 NeuronCores: batch 4096 -> 512 per core (4 btiles x 128).

Layout strategy (batch on partitions for all elementwise work; all engine
costs scale with free-dim size only, so elementwise tensors are shaped
[128 partitions, small free]):
  - x is pre-transposed HOST-side to xt [1600, 512] fp16 (row = 8*t + ch,
    ch 0:5 = x, ch 5 = 1.0 carrying fc1 bias + ones column): the DMA lands
    it directly in [128, 512] chunks (16 steps x 8ch on partitions), so no
    on-chip transposes for fc1.
  - fc1: per 4-step group, one matmul with a 4-copy block-diagonal
    stationary w1bd4 [32,128] at tile row position 32m -> psum [128, 512]
    (rows 32w:32w+21 = step's h1aug^T, pre-relu). One DVE copy to fp16
    SBUF storage P4_g. Relu is folded into the per-step staging copy (max).
  - Recurrence per step: stationary L = STG_par [53, 512] fp16
    (rows 0:32 h^T, 32:53 h1aug^T), moving = wcomb [53, 128];
    4 matmuls (one per 128-batch tile) -> gates G [128b, 512=4x128g] psum.
    Gate cols per btile: [f|i|g|o] x 32, with f,i,o columns pre-scaled 0.5.
    One tanh ACT -> t4; ts (t4+1)*0.5 -> P (sigmoids); 3 tts for
    c' = sig_f*c + sig_i*tanh_g; tanh ACT -> TC; tt -> H = sig_o*TC;
    2 PE transposes H -> psum; DVE copy -> STG_nextpar rows 0:32;
    Pool ts-copy (with relu max) h1aug_{t+1} -> STG_nextpar rows 32:53.
  - Two independent batch streams (btiles {0,1}, {2,3}) interleave their
    serial chains across the engines.
"""

import sys
import numpy as np
from contextlib import ExitStack

sys.path.insert(0, "/opt/trn_rl_repo")
sys.path.insert(0, "/opt/pypackages")

import concourse.bass as bass
import concourse.bacc as bacc
import concourse.tile as tile
import concourse.mybir as mybir
from concourse import bass_utils
from concourse.masks import make_identity

F32 = mybir.dt.float32
FP16 = mybir.dt.float16
AF = mybir.ActivationFunctionType
ALU = mybir.AluOpType

H = 32
B = 4096
T = 200
C8 = 8
NCORES = 8
BL = B // NCORES  # 512
NBT = BL // 128  # 4
NXT = (C8 * T + 127) // 128  # 13 xt chunks of [128, 512] (16 steps each)
NG = T // 4  # 50 fc1 groups of 4 steps

# gate blocks within a btile's 128 gate columns: [f, i, o, g]
_TORCH_BASE = {0: 32, 1: 0, 2: 96, 3: 64}  # f,i,o,g -> torch row base


def prep_consts(fc1_w, fc1_b, w_ih, w_hh, b_ih, b_hh, fc2_w, fc2_b):
    perm = np.zeros(4 * H, dtype=np.int64)
    scol = np.zeros(4 * H, dtype=np.float32)
    for col in range(4 * H):
        blk, j = col // H, col % H
        perm[col] = _TORCH_BASE[blk] + j
        scol[col] = 1.0 if blk == 3 else 0.5  # g unscaled, f/i/o halved
    # wch [32, 128]: h-weights; wcx4 [128, 128]: h1-weights + bias row,
    # replicated at partition offsets 32m so mmx fmap/weight offsets match
    wch = np.ascontiguousarray((scol[:, None] * w_hh[perm]).T)
    wcx = np.zeros((21, 128), np.float32)
    wcx[0:20] = (scol[:, None] * w_ih[perm]).T
    wcx[20] = scol * (b_ih + b_hh)[perm]
    wcx4 = np.zeros((128, 128), np.float32)
    for m in range(4):
        wcx4[32 * m : 32 * m + 21] = wcx
    # w1bd4 [128, 128]: 4 identical 32-row copies (m=0..3); within a copy,
    # row 8w+c (w=step-in-group, c=channel) -> cols 32w+r of step w's h1aug
    w1bd4 = np.zeros((128, 128), np.float32)
    for m in range(4):
        for w in range(4):
            for c in range(5):
                w1bd4[32 * m + 8 * w + c, 32 * w : 32 * w + 20] = fc1_w[:, c]
            w1bd4[32 * m + 8 * w + 5, 32 * w : 32 * w + 20] = fc1_b
            w1bd4[32 * m + 8 * w + 5, 32 * w + 20] = 1.0
    fc2wT = np.ascontiguousarray(fc2_w.T)  # [32, 2]
    f16 = np.float16
    wcomb = np.vstack([wch, wcx])  # [53, 128]
    return dict(wcomb=wcomb.astype(f16), w1bd4=w1bd4.astype(f16),
                fc2wT=fc2wT.astype(f16))


def emit(tc, outs, ins):
    nc = tc.nc
    ctx = ExitStack()
    out_d = outs["out"]  # [512, 2] f32

    consts = ctx.enter_context(tc.tile_pool(name="consts", bufs=1))
    ident = consts.tile([128, 128], FP16, tag="ident")
    make_identity(nc, ident[:])
    wcomb = consts.tile([53, 128], FP16, tag="wcomb")
    nc.sync.dma_start(wcomb[:], ins["wcomb"][:, :])
    w1bd4 = consts.tile([128, 128], FP16, tag="w1bd4")
    nc.sync.dma_start(w1bd4[:], ins["w1bd4"][:, :])
    fc2w = consts.tile([32, 2], FP16, tag="fc2w")
    nc.sync.dma_start(fc2w[:], ins["fc2wT"][:, :])

    # ---------------- pools ----------------
    xt_pool = ctx.enter_context(tc.tile_pool(name="xt", bufs=1))
    p4_pool = ctx.enter_context(tc.tile_pool(name="p4", bufs=1))
    st_pool = ctx.enter_context(tc.tile_pool(name="st", bufs=1))
    wk = ctx.enter_context(tc.tile_pool(name="wk", bufs=4))
    ps_g = ctx.enter_context(tc.tile_pool(name="psg", bufs=2, space="PSUM"))
    ps_f = ctx.enter_context(tc.tile_pool(name="psf", bufs=1, space="PSUM"))
    ps_h = ctx.enter_context(tc.tile_pool(name="psh", bufs=1, space="PSUM"))

    # xt chunks: DMA the host-transposed x straight in
    xtd = ins["xt"]  # [1664, 512] fp16 (13*128 rows; tail zero-padded)
    xtt = []
    for c in range(NXT):
        xc = xt_pool.tile([128, 512], FP16, tag=f"xt{c}", name=f"xt_{c}")
        nc.sync.dma_start(xc[:], xtd[128 * c : 128 * (c + 1), :])
        xtt.append(xc)

    # fc1 storage: 50 groups x [128, 512] fp16 (pre-relu h1aug^T, 4 steps)
    p4t = [p4_pool.tile([128, 512], FP16, tag=f"p4_{g}", name=f"p4_{g}")
           for g in range(NG)]

    def emit_fc1_group(g):
        c, m = g // 4, g % 4
        fps = ps_f.tile([128, 512], F32, tag="fps", name=f"fps_{g}")
        nc.tensor.matmul(fps[:], w1bd4[32 * m : 32 * (m + 1), :],
                         xtt[c][32 * m : 32 * (m + 1), :],
                         start=True, stop=True, tile_position=(32 * m, 0))
        nc.scalar.activation(p4t[g][:], fps[:], AF.Relu)

    # recurrence state [53, 512]: rows 0:32 h^T, rows 32:53 h1aug^T
    stg = [st_pool.tile([53, 512], FP16, tag=f"stg{p}", name=f"stg{p}")
           for p in range(2)]
    nc.vector.memset(stg[0][:], 0.0)
    Cst = [st_pool.tile([128, 64], FP16, tag=f"C{s}", name=f"Cst{s}")
           for s in range(2)]
    nc.vector.memset(Cst[0][:], 0.0)
    nc.vector.memset(Cst[1][:], 0.0)

    # prologue: fc1 groups 0..2, then stage h1aug_0 (both row bands)
    for g in range(3):
        emit_fc1_group(g)

    nc.vector.tensor_copy(stg[0][32:53, :], p4t[0][0:21, :])

    # ---------------- recurrence ----------------
    # State per stream: t4/TC/H tiles are looked up by (s) from these dicts,
    # written by one sub-phase and read by a later one. Emission is skewed:
    # stream b runs half a step behind stream a so their serial chains
    # interleave on the in-order engine queues.
    cur = [dict() for _ in range(2)]

    def p0_mm(t, s):  # gates matmul (PE)
        par = t % 2
        G = ps_g.tile([128, 256], F32, tag=f"G{s}", name=f"G{s}_{t}")
        Gv = G[:].rearrange("p (B two h) -> p B two h", two=2, h=32)
        for j in range(2):
            k = 2 * s + j
            nc.tensor.matmul(Gv[:, :, j, :],
                             stg[par][:, 128 * k : 128 * (k + 1)],
                             wcomb[:], start=True, stop=True,
                             tile_position=(0, 0))
        cur[s]["G"] = G

    def stage_h1aug(t):  # h1aug^T for step t+1 (DVE, off-chain tail)
        if t + 1 < T:
            parn = (t + 1) % 2
            g1, m1 = (t + 1) // 4, (t + 1) % 4
            nc.vector.tensor_copy(
                stg[parn][32:53, :],
                p4t[g1][32 * m1 : 32 * m1 + 21, :])

    def p1_tanh(t, s):  # gate nonlinearity (ACT), one instruction
        t4 = wk.tile([128, 256], FP16, tag=f"t4{s}", name=f"t4{s}_{t}")
        nc.scalar.activation(t4[:], cur[s]["G"][:], AF.Tanh)
        cur[s]["t4"] = t4

    def p2_cell(t, s):  # sigma-form cell: 1 ts (4x) + 3 tt (2x), [128,64]
        t4 = cur[s]["t4"]
        Cs = Cst[s][:]
        # SG = (t4_fio + 1) * 0.5 = [sig_f | sig_i | sig_o]  (cols 0:192)
        SG = wk.tile([128, 192], FP16, tag=f"SG{s}", name=f"SG{s}_{t}")
        nc.vector.tensor_scalar(SG[:], t4[:, 0:192], 1.0, 0.5,
                                ALU.add, ALU.mult)
        cur[s]["SG"] = SG
        U = wk.tile([128, 64], FP16, tag=f"U{s}", name=f"U{s}_{t}")
        nc.vector.tensor_tensor(U[:], SG[:, 0:64], Cs, ALU.mult)
        V = wk.tile([128, 64], FP16, tag=f"V{s}", name=f"V{s}_{t}")
        nc.vector.tensor_tensor(V[:], SG[:, 64:128], t4[:, 192:256],
                                ALU.mult)
        nc.vector.tensor_tensor(Cs, U[:], V[:], ALU.add)

    def p3_h(t, s):  # per-stream tanh(c) (ACT) + H (DVE) — streams decoupled
        TC = wk.tile([128, 64], FP16, tag=f"TC{s}", name=f"TC{s}_{t}")
        nc.scalar.activation(TC[:], Cst[s][:], AF.Tanh)
        Hs = wk.tile([128, 64], FP16, tag=f"H{s}", name=f"H{s}_{t}")
        nc.vector.tensor_tensor(Hs[:], cur[s]["SG"][:, 128:192], TC[:],
                                ALU.mult)
        cur[s]["H"] = Hs

    def p4_transpose(t, s):  # H -> H^T (PE)
        HT = ps_h.tile([32, 256], FP16, tag=f"HT{s}", name=f"HT{s}_{t}")
        for j in range(2):
            nc.tensor.transpose(HT[:, 128 * j : 128 * (j + 1)],
                                cur[s]["H"][:, 32 * j : 32 * (j + 1)],
                                ident[:])
        cur[s]["HT"] = HT

    def p5_stage(t, s):  # stage h^T (DVE)
        parn = (t + 1) % 2
        cols = slice(256 * s, 256 * (s + 1))
        nc.vector.tensor_copy(stg[parn][0:32, cols], cur[s]["HT"][:])

    for t in range(T):
        # Alternate which stream gets engine-queue priority each step so the
        # per-stage queuing penalty splits between streams instead of
        # stacking on one.
        f, g = (0, 1) if t % 2 == 0 else (1, 0)
        p0_mm(t, f)
        p1_tanh(t, f)
        p0_mm(t, g)
        p1_tanh(t, g)
        p2_cell(t, f)
        p3_h(t, f)
        p4_transpose(t, f)
        p5_stage(t, f)
        p2_cell(t, g)
        p3_h(t, g)
        p4_transpose(t, g)
        p5_stage(t, g)
        stage_h1aug(t)
        if t % 4 == 0 and t // 4 + 3 < NG:
            emit_fc1_group(t // 4 + 3)

    # ---------------- fc2 ----------------
    f2p = ps_f.tile([128, 8], F32, tag="f2p", name="f2p")
    for k in range(NBT):
        nc.tensor.matmul(f2p[:, 2 * k : 2 * k + 2],
                         stg[0][0:32, 128 * k : 128 * (k + 1)], fc2w[:],
                         start=True, stop=True, tile_position=(0, 0))
    f2s = wk.tile([128, 8], F32, tag="f2s", name="f2s")
    nc.vector.tensor_copy(f2s[:], f2p[:])
    for k in range(NBT):
        nc.sync.dma_start(out_d[128 * k : 128 * (k + 1), :],
                          f2s[:, 2 * k : 2 * k + 2])
    ctx.close()


_CACHE = {}


def _build():
    if "nc" in _CACHE:
        return _CACHE["nc"]
    nc = bacc.Bacc("TRN2", target_bir_lowering=False, debug=False,
                   enable_asserts=False, num_devices=NCORES)
    ins = {
        "xt": nc.dram_tensor("xt", [NXT * 128, BL], FP16,
                             kind="ExternalInput").ap(),
        "wcomb": nc.dram_tensor("wcomb", [53, 128], FP16,
                                kind="ExternalInput").ap(),
        "w1bd4": nc.dram_tensor("w1bd4", [128, 128], FP16,
                                kind="ExternalInput").ap(),
        "fc2wT": nc.dram_tensor("fc2wT", [32, 2], FP16,
                                kind="ExternalInput").ap(),
    }
    outs = {"out": nc.dram_tensor("out", [BL, 2], F32,
                                  kind="ExternalOutput").ap()}
    with tile.TileContext(nc) as tc:
        emit(tc, outs, ins)
    nc.compile()
    _CACHE["nc"] = nc
    return nc


def make_in_maps(x, fc1_w, fc1_b, w_ih, w_hh, b_ih, b_hh, fc2_w, fc2_b):
    consts = prep_consts(fc1_w, fc1_b, w_ih, w_hh, b_ih, b_hh, fc2_w, fc2_b)
    in_maps = []
    for c in range(NCORES):
        xs = x[c * BL : (c + 1) * BL]  # [512, 200, 5] f32
        x8 = np.zeros((BL, T, C8), np.float16)
        x8[:, :, 0:5] = xs
        x8[:, :, 5] = 1.0
        xt = np.zeros((NXT * 128, BL), np.float16)
        xt[0 : C8 * T] = x8.reshape(BL, C8 * T).T
        in_maps.append({"xt": np.ascontiguousarray(xt), **consts})
    return in_maps


def kernel(x, fc1_w, fc1_b, w_ih, w_hh, b_ih, b_hh, fc2_w, fc2_b,
           trace=False):
    x = np.asarray(x, np.float32)
    args = [np.asarray(a, np.float32)
            for a in (fc1_w, fc1_b, w_ih, w_hh, b_ih, b_hh, fc2_w, fc2_b)]
    nc = _build()
    in_maps = make_in_maps(x, *args)
    res = bass_utils.run_bass_kernel_spmd(
        nc, in_maps, core_ids=list(range(NCORES)), trace=trace)
    out = np.concatenate([r["out"] for r in res.results], axis=0)
    out = out + args[7][None, :]
    if trace:
        kernel.last_results = res
    return out.astype(np.float32)



# revision 32
# speedup vs baseline: 1.4975x; 1.0501x over previous
"""Trainium2 Bass kernel: fc1+relu -> LSTM(H=32, T=200) -> fc2 on last hidden.

Data parallel over 8 NeuronCores: batch 4096 -> 512 per core (4 btiles x 128).

Layout strategy (batch on partitions for all elementwise work; all engine
costs scale with free-dim size only, so elementwise tensors are shaped
[128 partitions, small free]):
  - x is pre-transposed HOST-side to xt [1600, 512] fp16 (row = 8*t + ch,
    ch 0:5 = x, ch 5 = 1.0 carrying fc1 bias + ones column): the DMA lands
    it directly in [128, 512] chunks (16 steps x 8ch on partitions), so no
    on-chip transposes for fc1.
  - fc1: per 4-step group, one matmul with a 4-copy block-diagonal
    stationary w1bd4 [32,128] at tile row position 32m -> psum [128, 512]
    (rows 32w:32w+21 = step's h1aug^T, pre-relu). One DVE copy to fp16
    SBUF storage P4_g. Relu is folded into the per-step staging copy (max).
  - Recurrence per step: stationary L = STG_par [53, 512] fp16
    (rows 0:32 h^T, 32:53 h1aug^T), moving = wcomb [53, 128];
    4 matmuls (one per 128-batch tile) -> gates G [128b, 512=4x128g] psum.
    Gate cols per btile: [f|i|g|o] x 32, with f,i,o columns pre-scaled 0.5.
    One tanh ACT -> t4; ts (t4+1)*0.5 -> P (sigmoids); 3 tts for
    c' = sig_f*c + sig_i*tanh_g; tanh ACT -> TC; tt -> H = sig_o*TC;
    2 PE transposes H -> psum; DVE copy -> STG_nextpar rows 0:32;
    Pool ts-copy (with relu max) h1aug_{t+1} -> STG_nextpar rows 32:53.
  - Two independent batch streams (btiles {0,1}, {2,3}) interleave their
    serial chains across the engines.
"""

import sys
import numpy as np
from contextlib import ExitStack

sys.path.insert(0, "/opt/trn_rl_repo")
sys.path.insert(0, "/opt/pypackages")

import concourse.bass as bass
import concourse.bacc as bacc
import concourse.tile as tile
import concourse.mybir as mybir
from concourse import bass_utils
from concourse.masks import make_identity

F32 = mybir.dt.float32
FP16 = mybir.dt.float16
AF = mybir.ActivationFunctionType
ALU = mybir.AluOpType

H = 32
B = 4096
T = 200
C8 = 8
NCORES = 8
BL = B // NCORES  # 512
NBT = BL // 128  # 4
NXT = (C8 * T + 127) // 128  # 13 xt chunks of [128, 512] (16 steps each)
NG = T // 4  # 50 fc1 groups of 4 steps

# gate blocks within a btile's 128 gate columns: [f, i, o, g]
_TORCH_BASE = {0: 32, 1: 0, 2: 96, 3: 64}  # f,i,o,g -> torch row base


def prep_consts(fc1_w, fc1_b, w_ih, w_hh, b_ih, b_hh, fc2_w, fc2_b):
    perm = np.zeros(4 * H, dtype=np.int64)
    scol = np.zeros(4 * H, dtype=np.float32)
    for col in range(4 * H):
        blk, j = col // H, col % H
        perm[col] = _TORCH_BASE[blk] + j
        scol[col] = 1.0 if blk == 3 else 0.5  # g unscaled, f/i/o halved
    # wch [32, 128]: h-weights; wcx4 [128, 128]: h1-weights + bias row,
    # replicated at partition offsets 32m so mmx fmap/weight offsets match
    wch = np.ascontiguousarray((scol[:, None] * w_hh[perm]).T)
    wcx = np.zeros((21, 128), np.float32)
    wcx[0:20] = (scol[:, None] * w_ih[perm]).T
    wcx[20] = scol * (b_ih + b_hh)[perm]
    wcx4 = np.zeros((128, 128), np.float32)
    for m in range(4):
        wcx4[32 * m : 32 * m + 21] = wcx
    # w1bd4 [128, 128]: 4 identical 32-row copies (m=0..3); within a copy,
    # row 8w+c (w=step-in-group, c=channel) -> cols 32w+r of step w's h1aug
    w1bd4 = np.zeros((128, 128), np.float32)
    for m in range(4):
        for w in range(4):
            for c in range(5):
                w1bd4[32 * m + 8 * w + c, 32 * w : 32 * w + 20] = fc1_w[:, c]
            w1bd4[32 * m + 8 * w + 5, 32 * w : 32 * w + 20] = fc1_b
            w1bd4[32 * m + 8 * w + 5, 32 * w + 20] = 1.0
    fc2wT = np.ascontiguousarray(fc2_w.T)  # [32, 2]
    f16 = np.float16
    wcomb = np.vstack([wch, wcx])  # [53, 128]
    return dict(wcomb=wcomb.astype(f16), w1bd4=w1bd4.astype(f16),
                fc2wT=fc2wT.astype(f16))


def emit(tc, outs, ins):
    nc = tc.nc
    ctx = ExitStack()
    out_d = outs["out"]  # [512, 2] f32

    consts = ctx.enter_context(tc.tile_pool(name="consts", bufs=1))
    ident = consts.tile([128, 128], FP16, tag="ident")
    make_identity(nc, ident[:])
    wcomb = consts.tile([53, 128], FP16, tag="wcomb")
    nc.sync.dma_start(wcomb[:], ins["wcomb"][:, :])
    w1bd4 = consts.tile([128, 128], FP16, tag="w1bd4")
    nc.sync.dma_start(w1bd4[:], ins["w1bd4"][:, :])
    fc2w = consts.tile([32, 2], FP16, tag="fc2w")
    nc.sync.dma_start(fc2w[:], ins["fc2wT"][:, :])

    # ---------------- pools ----------------
    xt_pool = ctx.enter_context(tc.tile_pool(name="xt", bufs=1))
    p4_pool = ctx.enter_context(tc.tile_pool(name="p4", bufs=1))
    st_pool = ctx.enter_context(tc.tile_pool(name="st", bufs=1))
    wk = ctx.enter_context(tc.tile_pool(name="wk", bufs=4))
    ps_g = ctx.enter_context(tc.tile_pool(name="psg", bufs=2, space="PSUM"))
    ps_f = ctx.enter_context(tc.tile_pool(name="psf", bufs=1, space="PSUM"))
    ps_h = ctx.enter_context(tc.tile_pool(name="psh", bufs=1, space="PSUM"))

    # xt chunks: DMA the host-transposed x straight in
    xtd = ins["xt"]  # [1664, 512] fp16 (13*128 rows; tail zero-padded)
    xtt = []
    for c in range(NXT):
        xc = xt_pool.tile([128, 512], FP16, tag=f"xt{c}", name=f"xt_{c}")
        nc.sync.dma_start(xc[:], xtd[128 * c : 128 * (c + 1), :])
        xtt.append(xc)

    # fc1 storage: 50 groups x [128, 512] fp16 (pre-relu h1aug^T, 4 steps)
    p4t = [p4_pool.tile([128, 512], FP16, tag=f"p4_{g}", name=f"p4_{g}")
           for g in range(NG)]

    def emit_fc1_group(g):
        c, m = g // 4, g % 4
        fps = ps_f.tile([128, 512], F32, tag="fps", name=f"fps_{g}")
        nc.tensor.matmul(fps[:], w1bd4[32 * m : 32 * (m + 1), :],
                         xtt[c][32 * m : 32 * (m + 1), :],
                         start=True, stop=True, tile_position=(32 * m, 0))
        nc.scalar.activation(p4t[g][:], fps[:], AF.Relu)

    # recurrence state [53, 512]: rows 0:32 h^T, rows 32:53 h1aug^T
    stg = [st_pool.tile([53, 512], FP16, tag=f"stg{p}", name=f"stg{p}")
           for p in range(2)]
    nc.vector.memset(stg[0][:], 0.0)
    Cst = [st_pool.tile([128, 64], FP16, tag=f"C{s}", name=f"Cst{s}")
           for s in range(2)]
    nc.vector.memset(Cst[0][:], 0.0)
    nc.vector.memset(Cst[1][:], 0.0)

    # prologue: fc1 groups 0..2, then stage h1aug_0 (both row bands)
    for g in range(3):
        emit_fc1_group(g)

    nc.vector.tensor_copy(stg[0][32:53, :], p4t[0][0:21, :])

    # ---------------- recurrence ----------------
    # State per stream: t4/TC/H tiles are looked up by (s) from these dicts,
    # written by one sub-phase and read by a later one. Emission is skewed:
    # stream b runs half a step behind stream a so their serial chains
    # interleave on the in-order engine queues.
    cur = [dict() for _ in range(2)]

    def p0_mm(t, s):  # gates matmul (PE)
        par = t % 2
        G = ps_g.tile([128, 256], F32, tag=f"G{s}", name=f"G{s}_{t}")
        Gv = G[:].rearrange("p (B two h) -> p B two h", two=2, h=32)
        for j in range(2):
            k = 2 * s + j
            nc.tensor.matmul(Gv[:, :, j, :],
                             stg[par][:, 128 * k : 128 * (k + 1)],
                             wcomb[:], start=True, stop=True,
                             tile_position=(0, 0))
        cur[s]["G"] = G

    def stage_h1aug(t):  # h1aug^T for step t+1 (DVE, off-chain tail)
        if t + 1 < T:
            parn = (t + 1) % 2
            g1, m1 = (t + 1) // 4, (t + 1) % 4
            nc.sync.dma_start(
                stg[parn][32:53, :],
                p4t[g1][32 * m1 : 32 * m1 + 21, :])

    def p1_tanh(t, s):  # gate nonlinearity (ACT), one instruction
        t4 = wk.tile([128, 256], FP16, tag=f"t4{s}", name=f"t4{s}_{t}")
        nc.scalar.activation(t4[:], cur[s]["G"][:], AF.Tanh)
        cur[s]["t4"] = t4

    def p2_cell(t, s):  # sigma-form cell: 1 ts (4x) + 3 tt (2x), [128,64]
        t4 = cur[s]["t4"]
        Cs = Cst[s][:]
        # SG = (t4_fio + 1) * 0.5 = [sig_f | sig_i | sig_o]  (cols 0:192)
        SG = wk.tile([128, 192], FP16, tag=f"SG{s}", name=f"SG{s}_{t}")
        nc.vector.tensor_scalar(SG[:], t4[:, 0:192], 1.0, 0.5,
                                ALU.add, ALU.mult)
        cur[s]["SG"] = SG
        U = wk.tile([128, 64], FP16, tag=f"U{s}", name=f"U{s}_{t}")
        nc.vector.tensor_tensor(U[:], SG[:, 0:64], Cs, ALU.mult)
        V = wk.tile([128, 64], FP16, tag=f"V{s}", name=f"V{s}_{t}")
        nc.vector.tensor_tensor(V[:], SG[:, 64:128], t4[:, 192:256],
                                ALU.mult)
        nc.vector.tensor_tensor(Cs, U[:], V[:], ALU.add)

    def p3_h(t, s):  # per-stream tanh(c) (ACT) + H (DVE) — streams decoupled
        TC = wk.tile([128, 64], FP16, tag=f"TC{s}", name=f"TC{s}_{t}")
        nc.scalar.activation(TC[:], Cst[s][:], AF.Tanh)
        Hs = wk.tile([128, 64], FP16, tag=f"H{s}", name=f"H{s}_{t}")
        nc.vector.tensor_tensor(Hs[:], cur[s]["SG"][:, 128:192], TC[:],
                                ALU.mult)
        cur[s]["H"] = Hs

    def p4_transpose(t, s):  # H -> H^T (PE)
        HT = ps_h.tile([32, 256], FP16, tag=f"HT{s}", name=f"HT{s}_{t}")
        for j in range(2):
            nc.tensor.transpose(HT[:, 128 * j : 128 * (j + 1)],
                                cur[s]["H"][:, 32 * j : 32 * (j + 1)],
                                ident[:])
        cur[s]["HT"] = HT

    def p5_stage(t, s):  # stage h^T (DVE)
        parn = (t + 1) % 2
        cols = slice(256 * s, 256 * (s + 1))
        nc.vector.tensor_copy(stg[parn][0:32, cols], cur[s]["HT"][:])

    for t in range(T):
        # Alternate which stream gets engine-queue priority each step so the
        # per-stage queuing penalty splits between streams instead of
        # stacking on one.
        f, g = (0, 1) if t % 2 == 0 else (1, 0)
        p0_mm(t, f)
        p1_tanh(t, f)
        p0_mm(t, g)
        p1_tanh(t, g)
        p2_cell(t, f)
        p3_h(t, f)
        p4_transpose(t, f)
        p5_stage(t, f)
        p2_cell(t, g)
        p3_h(t, g)
        p4_transpose(t, g)
        p5_stage(t, g)
        stage_h1aug(t)
        if t % 4 == 0 and t // 4 + 3 < NG:
            emit_fc1_group(t // 4 + 3)

    # ---------------- fc2 ----------------
    f2p = ps_f.tile([128, 8], F32, tag="f2p", name="f2p")
    for k in range(NBT):
        nc.tensor.matmul(f2p[:, 2 * k : 2 * k + 2],
                         stg[0][0:32, 128 * k : 128 * (k + 1)], fc2w[:],
                         start=True, stop=True, tile_position=(0, 0))
    f2s = wk.tile([128, 8], F32, tag="f2s", name="f2s")
    nc.vector.tensor_copy(f2s[:], f2p[:])
    for k in range(NBT):
        nc.sync.dma_start(out_d[128 * k : 128 * (k + 1), :],
                          f2s[:, 2 * k : 2 * k + 2])
    ctx.close()


_CACHE = {}


def _build():
    if "nc" in _CACHE:
        return _CACHE["nc"]
    nc = bacc.Bacc("TRN2", target_bir_lowering=False, debug=False,
                   enable_asserts=False, num_devices=NCORES)
    ins = {
        "xt": nc.dram_tensor("xt", [NXT * 128, BL], FP16,
                             kind="ExternalInput").ap(),
        "wcomb": nc.dram_tensor("wcomb", [53, 128], FP16,
                                kind="ExternalInput").ap(),
        "w1bd4": nc.dram_tensor("w1bd4", [128, 128], FP16,
                                kind="ExternalInput").ap(),
        "fc2wT": nc.dram_tensor("fc2wT", [32, 2], FP16,
                                kind="ExternalInput").ap(),
    }
    outs = {"out": nc.dram_tensor("out", [BL, 2], F32,
                                  kind="ExternalOutput").ap()}
    with tile.TileContext(nc) as tc:
        emit(tc, outs, ins)
    nc.compile()
    _CACHE["nc"] = nc
    return nc


def make_in_maps(x, fc1_w, fc1_b, w_ih, w_hh, b_ih, b_hh, fc2_w, fc2_b):
    consts = prep_consts(fc1_w, fc1_b, w_ih, w_hh, b_ih, b_hh, fc2_w, fc2_b)
    in_maps = []
    for c in range(NCORES):
        xs = x[c * BL : (c + 1) * BL]  # [512, 200, 5] f32
        x8 = np.zeros((BL, T, C8), np.float16)
        x8[:, :, 0:5] = xs
        x8[:, :, 5] = 1.0
        xt = np.zeros((NXT * 128, BL), np.float16)
        xt[0 : C8 * T] = x8.reshape(BL, C8 * T).T
        in_maps.append({"xt": np.ascontiguousarray(xt), **consts})
    return in_maps


def kernel(x, fc1_w, fc1_b, w_ih, w_hh, b_ih, b_hh, fc2_w, fc2_b,
           trace=False):
    x = np.asarray(x, np.float32)
    args = [np.asarray(a, np.float32)
            for a in (fc1_w, fc1_b, w_ih, w_hh, b_ih, b_hh, fc2_w, fc2_b)]
    nc = _build()
    in_maps = make_in_maps(x, *args)
    res = bass_utils.run_bass_kernel_spmd(
        nc, in_maps, core_ids=list(range(NCORES)), trace=trace)
    out = np.concatenate([r["out"] for r in res.results], axis=0)
    out = out + args[7][None, :]
    if trace:
        kernel.last_results = res
    return out.astype(np.float32)

